# revision 7
# baseline (speedup 1.0000x reference)
"""DecodeDetections kernel for Trainium2 (Bass/Tile), 8-core data-parallel.

Full input y_pred [64, 8732, 33] f32 -> output [64, 200, 6] f32.
Each of the 8 NeuronCores handles 8 batch items ("tokens"):
  decode SSD boxes, per-class scores, exact top-200 (jax top_k tie rules).

Pipeline per core (tokens t=0..7 on partition groups [16t, 16t+16)):
  1. DMA raw rows into SBUF [128, 546*33] (box-blocked per partition).
  2. DVE strided copy -> scores S [128, 10920]  (v = i*10920 + j*20 + cls).
  3. 3x gpsimd topk (vocab 58240) -> per-chunk top-256 (values+indices).
  4. gpsimd topk on the 768 chunk-winners (padded vocab 57344) -> exact
     per-token top-256 values, sorted ascending.
  5. Stage chunk-topk indices to DRAM; per-winner indirect gather of v.
  6. v -> (cls, n); indirect gather of the winners' 33-ch rows; decode boxes.
  7. Exact rank (value desc, m=cls*8732+n asc; +-2 tie window) and
     indirect scatter of rows [class_id, conf, xmin, ymin, xmax, ymax]
     to out[t*200 + rank] with bounds_check dropping rank >= 200.
"""

import os
import sys

for _p in ("/opt/trn_rl_repo", "/root/.axon_site/_ro/trn_rl_repo"):
    if os.path.isdir(_p) and _p not in sys.path:
        sys.path.insert(0, _p)

import numpy as np

import concourse.bass as bass
import concourse.bacc as bacc
import concourse.bass_isa as bass_isa
import concourse.mybir as mybir
import concourse.tile as tile
from concourse.bass_utils import run_bass_kernel_spmd

# problem constants
B = 64
NBOX = 8732
NCH = 33
NCLS = 20          # foreground classes (channels 1..20)
TOPK = 200
NCORES = 8
TPC = 8            # tokens (batch items) per core

NB = 546           # boxes per partition (546*16 = 8736 >= 8732)
NBP = 8736         # padded boxes per token in DRAM
RAWC = NB * NCH    # 18018
SCOLS = NB * NCLS  # 10920 score cols per partition
CHUNK = 58240      # stage-1 topk vocab (SCOLS/3 * 16)
CCOLS = CHUNK // 16  # 3640
V2 = 57344         # stage-2 topk vocab
V2C = V2 // 16     # 3584
IMG = 512.0


def _topk(nc, out_ap, in_ap, tokens, vocab, k=256):
    _in = nc.gpsimd.lower_ap(in_ap, for_isa=True)
    _out = nc.gpsimd.lower_ap(out_ap, for_isa=True)
    return nc.gpsimd.add_instruction(
        bass_isa.InstTopk(name=f"I-{nc.next_id()}", ins=[_in], outs=[_out],
                          _tokens=tokens, _n=vocab, _k=k))


class _Helper:
    """Float-exact integer div/mod on [128, W] f32 tiles."""

    def __init__(self, nc, pool, w):
        self.nc, self.pool, self.w = nc, pool, w
        self.t1 = pool.tile([128, w], mybir.dt.float32, name="hlp_t1")
        self.ti = pool.tile([128, w], mybir.dt.int32, name="hlp_ti")
        self.t2 = pool.tile([128, w], mybir.dt.float32, name="hlp_t2")

    def fdiv(self, out, in_, d):
        """out = floor(in_/d) for integer-valued f32 in_ >= 0 (exact)."""
        nc = self.nc
        nc.vector.tensor_scalar(self.t1[:], in_, float((1 + 2.0 ** -20) / d),
                                scalar2=None, op0=mybir.AluOpType.mult)
        nc.vector.tensor_copy(self.ti[:], self.t1[:])   # f32 -> i32
        nc.vector.tensor_copy(out, self.ti[:])          # i32 -> f32
        nc.vector.tensor_scalar(self.t1[:], out, float(d),
                                scalar2=None, op0=mybir.AluOpType.mult)
        nc.vector.tensor_tensor(self.t2[:], self.t1[:], in_,
                                op=mybir.AluOpType.is_gt)
        nc.vector.tensor_tensor(out, out, self.t2[:],
                                op=mybir.AluOpType.subtract)

    def fmod(self, out, in_, quot, d):
        """out = in_ - quot*d (exact)."""
        nc = self.nc
        nc.vector.tensor_scalar(self.t1[:], quot, float(d),
                                scalar2=None, op0=mybir.AluOpType.mult)
        nc.vector.tensor_tensor(out, in_, self.t1[:],
                                op=mybir.AluOpType.subtract)


def build_kernel():
    nc = bacc.Bacc("TRN2", target_bir_lowering=False, debug=False)
    y = nc.dram_tensor("y", [TPC * NBP, NCH], mybir.dt.float32,
                       kind="ExternalInput")
    out = nc.dram_tensor("out", [TPC * TOPK, 6], mybir.dt.float32,
                         kind="ExternalOutput")

    with tile.TileContext(nc) as tc:
        with tc.tile_pool(name="sbuf", bufs=1) as pool, \
             tc.tile_pool(name="dram", bufs=1, space="DRAM") as dpool:

            raw = pool.tile([128, RAWC], mybir.dt.float32)
            S = pool.tile([128, SCOLS], mybir.dt.float32)
            tk1 = pool.tile([128, 96], mybir.dt.uint32)
            pad2 = pool.tile([128, V2C], mybir.dt.float32)
            tk2 = pool.tile([128, 32], mybir.dt.uint32)

            yv = y[:].rearrange("(t i b) c -> t i (b c)", t=TPC, i=16)

            nc.gpsimd.memset(pad2[:], 0.0)

            # 3 box-range chunks: DMA -> score copy -> chunk topk
            NBC = NB // 3  # 182 boxes per chunk
            for j in range(3):
                with nc.named_scope(f"load{j}"):
                    nc.sync.dma_start(
                        raw[:, j * NBC * NCH:(j + 1) * NBC * NCH],
                        yv[:, :, j * NBC * NCH:(j + 1) * NBC * NCH])
                with nc.named_scope(f"scopy{j}"):
                    nc.vector.tensor_copy(
                        S[:, j * CCOLS:(j + 1) * CCOLS].rearrange(
                            "p (b c) -> p b c", c=NCLS),
                        raw[:].rearrange("p (b c) -> p b c", c=NCH)[
                            :, j * NBC:(j + 1) * NBC, 1:1 + NCLS])
                with nc.named_scope(f"topk{j}"):
                    _topk(nc, tk1[:, j * 32:(j + 1) * 32],
                          S[:, j * CCOLS:(j + 1) * CCOLS],
                          tokens=TPC, vocab=CHUNK)

            # stage-2: exact top-256 of the 768 chunk winners per token
            nc.vector.tensor_copy(
                pad2[:, 0:48].rearrange("p (j c) -> p j c", j=3),
                tk1[:].bitcast(mybir.dt.float32).rearrange(
                    "p (j b) -> p j b", j=3)[:, :, 0:16])
            with nc.named_scope("topk4"):
                _topk(nc, tk2[:], pad2[:], tokens=TPC, vocab=V2)

            # stage chunk-topk indices to DRAM for the v-lookup
            tk1d = dpool.tile([128 * 96, 1], mybir.dt.uint32)
            nc.sync.dma_start(
                tk1d[:].rearrange("(p c) o -> p (c o)", p=128), tk1[:])

            h = _Helper(nc, pool, 16)

            # per-partition constants: t = p // 16
            pidx = pool.tile([128, 1], mybir.dt.int32)
            nc.gpsimd.iota(pidx[:], pattern=[[0, 1]], base=0,
                           channel_multiplier=1)
            pf = pool.tile([128, 1], mybir.dt.float32)
            nc.vector.tensor_copy(pf[:], pidx[:])
            h1 = _Helper(nc, pool, 1)
            tf = pool.tile([128, 1], mybir.dt.float32)
            h1.fdiv(tf[:], pf[:], 16)

            # winners: q2 + value
            q2f = pool.tile([128, 16], mybir.dt.float32)
            nc.vector.tensor_copy(q2f[:], tk2[:, 16:32])  # u32 -> f32
            conf = pool.tile([128, 16], mybir.dt.float32)
            nc.vector.tensor_copy(conf[:], tk2[:, 0:16].bitcast(mybir.dt.float32))

            i2 = pool.tile([128, 16], mybir.dt.float32)
            c2 = pool.tile([128, 16], mybir.dt.float32)
            jj = pool.tile([128, 16], mybir.dt.float32)
            cc = pool.tile([128, 16], mybir.dt.float32)
            h.fdiv(i2[:], q2f[:], V2C)
            h.fmod(c2[:], q2f[:], i2[:], V2C)
            h.fdiv(jj[:], c2[:], 16)
            h.fmod(cc[:], c2[:], jj[:], 16)

            # F = (16t + i2)*96 + 32j + 16 + c
            F = pool.tile([128, 16], mybir.dt.float32)
            nc.vector.tensor_scalar(F[:], i2[:], 96.0, scalar2=None,
                                    op0=mybir.AluOpType.mult)
            tmp = pool.tile([128, 16], mybir.dt.float32)
            nc.vector.tensor_scalar(tmp[:], jj[:], 32.0, scalar2=16.0,
                                    op0=mybir.AluOpType.mult,
                                    op1=mybir.AluOpType.add)
            nc.vector.tensor_tensor(F[:], F[:], tmp[:], op=mybir.AluOpType.add)
            nc.vector.tensor_tensor(F[:], F[:], cc[:], op=mybir.AluOpType.add)
            t1536 = pool.tile([128, 1], mybir.dt.float32)
            nc.vector.tensor_scalar(t1536[:], tf[:], 1536.0, scalar2=None,
                                    op0=mybir.AluOpType.mult)
            nc.vector.tensor_scalar(F[:], F[:], t1536[:, 0:1], scalar2=None,
                                    op0=mybir.AluOpType.add)
            Fu = pool.tile([128, 16], mybir.dt.uint32)
            nc.vector.tensor_copy(Fu[:], F[:])

            # gather v (chunk-local score index) per winner
            vidx = pool.tile([128, 16], mybir.dt.uint32)
            with nc.named_scope("gather_v"):
                for k in range(16):
                    nc.gpsimd.indirect_dma_start(
                        out=vidx[:, k:k + 1], out_offset=None, in_=tk1d[:],
                        in_offset=bass.IndirectOffsetOnAxis(
                            ap=Fu[:, k:k + 1], axis=0),
                        bounds_check=128 * 96 - 1, oob_is_err=False)

            # v -> (i, jb, cls); chunk offset: v_global = j*?? -- v is local to
            # its chunk's S columns: col = v % CCOLS maps to (b, c) within the
            # chunk; true score col = j*CCOLS + col, flat v = i*SCOLS + scol.
            vf = pool.tile([128, 16], mybir.dt.float32)
            nc.vector.tensor_copy(vf[:], vidx[:])
            iv = pool.tile([128, 16], mybir.dt.float32)
            rv = pool.tile([128, 16], mybir.dt.float32)
            h.fdiv(iv[:], vf[:], CCOLS)
            h.fmod(rv[:], vf[:], iv[:], CCOLS)
            # scol = j*CCOLS + rv ; box-in-partition jb = scol // 20, cls = scol % 20
            scol = pool.tile([128, 16], mybir.dt.float32)
            nc.vector.tensor_scalar(tmp[:], jj[:], float(CCOLS), scalar2=None,
                                    op0=mybir.AluOpType.mult)
            nc.vector.tensor_tensor(scol[:], rv[:], tmp[:],
                                    op=mybir.AluOpType.add)
            jb = pool.tile([128, 16], mybir.dt.float32)
            cls = pool.tile([128, 16], mybir.dt.float32)
            h.fdiv(jb[:], scol[:], NCLS)
            h.fmod(cls[:], scol[:], jb[:], NCLS)
            # n = iv*546 + jb ; m = cls*8732 + n ; row = t*8736 + n
            n_ = pool.tile([128, 16], mybir.dt.float32)
            nc.vector.tensor_scalar(n_[:], iv[:], float(NB), scalar2=None,
                                    op0=mybir.AluOpType.mult)
            nc.vector.tensor_tensor(n_[:], n_[:], jb[:], op=mybir.AluOpType.add)
            m_ = pool.tile([128, 16], mybir.dt.float32)
            nc.vector.tensor_scalar(m_[:], cls[:], float(NBOX), scalar2=None,
                                    op0=mybir.AluOpType.mult)
            nc.vector.tensor_tensor(m_[:], m_[:], n_[:], op=mybir.AluOpType.add)
            row = pool.tile([128, 16], mybir.dt.float32)
            t8736 = pool.tile([128, 1], mybir.dt.float32)
            nc.vector.tensor_scalar(t8736[:], tf[:], float(NBP), scalar2=None,
                                    op0=mybir.AluOpType.mult)
            nc.vector.tensor_scalar(row[:], n_[:], t8736[:, 0:1], scalar2=None,
                                    op0=mybir.AluOpType.add)
            rowu = pool.tile([128, 16], mybir.dt.uint32)
            nc.vector.tensor_copy(rowu[:], row[:])

            # gather winner rows (33 ch each)
            enc = pool.tile([128, 16, NCH], mybir.dt.float32)
            with nc.named_scope("gather_rows"):
                for k in range(16):
                    nc.gpsimd.indirect_dma_start(
                        out=enc[:, k, :], out_offset=None, in_=y[:],
                        in_offset=bass.IndirectOffsetOnAxis(
                            ap=rowu[:, k:k + 1], axis=0),
                        bounds_check=TPC * NBP - 1, oob_is_err=False)

            # decode boxes on [128, 16] strided slices (enc ch 21..32)
            def ch(k):
                return enc[:, :, 21 + k]

            cx = pool.tile([128, 16], mybir.dt.float32)
            cy = pool.tile([128, 16], mybir.dt.float32)
            we = pool.tile([128, 16], mybir.dt.float32)
            he = pool.tile([128, 16], mybir.dt.float32)
            nc.vector.tensor_tensor(cx[:], ch(0), ch(8), op=mybir.AluOpType.mult)
            nc.vector.tensor_tensor(cx[:], cx[:], ch(4 + 2), op=mybir.AluOpType.mult)
            nc.vector.tensor_tensor(cx[:], cx[:], ch(4), op=mybir.AluOpType.add)
            nc.vector.tensor_tensor(cy[:], ch(1), ch(9), op=mybir.AluOpType.mult)
            nc.vector.tensor_tensor(cy[:], cy[:], ch(4 + 3), op=mybir.AluOpType.mult)
            nc.vector.tensor_tensor(cy[:], cy[:], ch(5), op=mybir.AluOpType.add)
            nc.vector.tensor_tensor(we[:], ch(2), ch(10), op=mybir.AluOpType.mult)
            nc.vector.tensor_tensor(he[:], ch(3), ch(11), op=mybir.AluOpType.mult)
            # exp(x) for x in [0,1) via degree-10 Taylor Horner (~1 ulp;
            # ACT's table exp is only ~2.5e-5 relative)
            import math as _math
            EXP_C = [1.0 / _math.factorial(kk) for kk in range(11)]
            xe = pool.tile([128, 32], mybir.dt.float32)
            nc.vector.tensor_copy(xe[:, 0:16], we[:])
            nc.vector.tensor_copy(xe[:, 16:32], he[:])
            acc = pool.tile([128, 32], mybir.dt.float32)
            nc.vector.memset(acc[:], EXP_C[10])
            for kk in range(9, -1, -1):
                nc.vector.tensor_tensor(acc[:], acc[:], xe[:],
                                        op=mybir.AluOpType.mult)
                nc.vector.tensor_scalar(acc[:], acc[:], EXP_C[kk],
                                        scalar2=None, op0=mybir.AluOpType.add)
            nc.vector.tensor_tensor(we[:], acc[:, 0:16], ch(6),
                                    op=mybir.AluOpType.mult)
            nc.vector.tensor_tensor(he[:], acc[:, 16:32], ch(7),
                                    op=mybir.AluOpType.mult)

            rows6 = pool.tile([128, 16, 6], mybir.dt.float32)
            # class_id = cls + 1 ; conf
            nc.vector.tensor_scalar(rows6[:, :, 0], cls[:], 1.0, scalar2=None,
                                    op0=mybir.AluOpType.add)
            nc.vector.tensor_copy(rows6[:, :, 1], conf[:])
            cxs = pool.tile([128, 16], mybir.dt.float32)
            whs = pool.tile([128, 16], mybir.dt.float32)
            # xmin/xmax = cx*512 -+ we*256
            nc.vector.tensor_scalar(cxs[:], cx[:], IMG, scalar2=None,
                                    op0=mybir.AluOpType.mult)
            nc.vector.tensor_scalar(whs[:], we[:], IMG / 2, scalar2=None,
                                    op0=mybir.AluOpType.mult)
            nc.vector.tensor_tensor(rows6[:, :, 2], cxs[:], whs[:],
                                    op=mybir.AluOpType.subtract)
            nc.vector.tensor_tensor(rows6[:, :, 4], cxs[:], whs[:],
                                    op=mybir.AluOpType.add)
            nc.vector.tensor_scalar(cxs[:], cy[:], IMG, scalar2=None,
                                    op0=mybir.AluOpType.mult)
            nc.vector.tensor_scalar(whs[:], he[:], IMG / 2, scalar2=None,
                                    op0=mybir.AluOpType.mult)
            nc.vector.tensor_tensor(rows6[:, :, 3], cxs[:], whs[:],
                                    op=mybir.AluOpType.subtract)
            nc.vector.tensor_tensor(rows6[:, :, 5], cxs[:], whs[:],
                                    op=mybir.AluOpType.add)

            # ---- exact rank with +-2 tie window on [8, 260] layout ----
            W = 2
            Vs = pool.tile([8, 256 + 2 * W], mybir.dt.float32)
            Ms = pool.tile([8, 256 + 2 * W], mybir.dt.float32)
            nc.vector.memset(Vs[:], -1.0)
            nc.vector.memset(Ms[:], 0.0)
            # relayout [128,16] -> [8,256] via DRAM staging (partition change)
            vmd = dpool.tile([2048, 2], mybir.dt.float32)
            nc.sync.dma_start(
                vmd[:, 0:1].rearrange("(p c) o -> p (c o)", p=128), conf[:])
            nc.sync.dma_start(
                vmd[:, 1:2].rearrange("(p c) o -> p (c o)", p=128), m_[:])
            nc.sync.dma_start(
                Vs[0:8, W:W + 256],
                vmd[:, 0:1].rearrange("(t q) o -> t (q o)", t=8))
            nc.sync.dma_start(
                Ms[0:8, W:W + 256],
                vmd[:, 1:2].rearrange("(t q) o -> t (q o)", t=8))

            Vc = Vs[:, W:W + 256]
            Mc = Ms[:, W:W + 256]
            rnk = pool.tile([8, 256], mybir.dt.float32)
            ri = pool.tile([8, 256], mybir.dt.int32)
            nc.gpsimd.iota(ri[:], pattern=[[-1, 256]], base=255,
                           channel_multiplier=0)
            nc.vector.tensor_copy(rnk[:], ri[:])  # 255 - q
            eq = pool.tile([8, 256], mybir.dt.float32)
            lt = pool.tile([8, 256], mybir.dt.float32)
            for d in (1, 2, -1, -2):
                Vd = Vs[:, W + d:W + d + 256]
                Md = Ms[:, W + d:W + d + 256]
                nc.vector.tensor_tensor(eq[:], Vc, Vd, op=mybir.AluOpType.is_equal)
                if d > 0:
                    # u term: subtract equal-above count
                    nc.vector.tensor_tensor(rnk[:], rnk[:], eq[:],
                                            op=mybir.AluOpType.subtract)
                nc.vector.tensor_tensor(lt[:], Md, Mc, op=mybir.AluOpType.is_lt)
                nc.vector.tensor_tensor(lt[:], lt[:], eq[:],
                                        op=mybir.AluOpType.mult)
                nc.vector.tensor_tensor(rnk[:], rnk[:], lt[:],
                                        op=mybir.AluOpType.add)

            # route rank back to [128, 16] winner layout via DRAM
            rnkd = dpool.tile([2048, 1], mybir.dt.float32)
            nc.sync.dma_start(
                rnkd[:].rearrange("(t q) o -> t (q o)", t=8), rnk[:])
            rnk128 = pool.tile([128, 16], mybir.dt.float32)
            nc.sync.dma_start(
                rnk128[:], rnkd[:].rearrange("(p c) o -> p (c o)", p=128))

            # dest = t*200 + rank (drop rank >= 200 via bounds_check)
            dest = pool.tile([128, 16], mybir.dt.float32)
            t200 = pool.tile([128, 1], mybir.dt.float32)
            nc.vector.tensor_scalar(t200[:], tf[:], 200.0, scalar2=None,
                                    op0=mybir.AluOpType.mult)
            nc.vector.tensor_scalar(dest[:], rnk128[:], t200[:, 0:1],
                                    scalar2=None, op0=mybir.AluOpType.add)
            big = pool.tile([128, 16], mybir.dt.float32)
            nc.vector.tensor_scalar(big[:], rnk128[:], 199.5, scalar2=1e6,
                                    op0=mybir.AluOpType.is_gt,
                                    op1=mybir.AluOpType.mult)
            nc.vector.tensor_tensor(dest[:], dest[:], big[:],
                                    op=mybir.AluOpType.add)
            destu = pool.tile([128, 16], mybir.dt.uint32)
            nc.vector.tensor_copy(destu[:], dest[:])

            with nc.named_scope("scatter_rows"):
                for k in range(16):
                    nc.gpsimd.indirect_dma_start(
                        out=out[:],
                        out_offset=bass.IndirectOffsetOnAxis(
                            ap=destu[:, k:k + 1], axis=0),
                        in_=rows6[:, k, :], in_offset=None,
                        bounds_check=TPC * TOPK - 1, oob_is_err=False)

    nc.finalize()
    return nc


_NC = None


def kernel(y_pred: np.ndarray, _trace: bool = False) -> np.ndarray:
    global _NC
    y_pred = np.asarray(y_pred, dtype=np.float32)
    assert y_pred.shape == (B, NBOX, NCH)
    if _NC is None:
        _NC = build_kernel()
    in_maps = []
    for c in range(NCORES):
        sl = y_pred[c * TPC:(c + 1) * TPC]          # [8, 8732, 33]
        ypad = np.zeros((TPC, NBP, NCH), np.float32)
        ypad[:, :NBOX] = sl
        in_maps.append({"y": ypad.reshape(TPC * NBP, NCH)})
    res = run_bass_kernel_spmd(_NC, in_maps, core_ids=list(range(NCORES)),
                               trace=_trace)
    kernel._last_results = res
    outs = [r["out"].reshape(TPC, TOPK, 6) for r in res.results]
    return np.concatenate(outs, axis=0)
